# revision 1
# baseline (speedup 1.0000x reference)
"""Trainium2 Bass kernel for the ChiSq (histogram_binning) problem.

Pipeline per core (128 of the 1024 (batch,channel) rows, pure data parallel):
  1. rfft(16384) of template & strain via 2-stage Cooley-Tukey matmul FFT
     (128x128 DFT stages on the PE array + elementwise twiddle).
  2. ph = c*|Ht|^2, cross = c*Re(conj(Ht) Hs) on the [k2, (row,k1)] grid.
  3. Unflatten to [row, k] layout, cumsum (tensor_tensor_scan), threshold
     masked sums -> per-bin SNR -> chi-square.
"""
import numpy as np
from contextlib import ExitStack

import concourse.bass as bass
import concourse.tile as tile
from concourse import bacc, mybir
from concourse.bass_utils import run_bass_kernel_spmd

F32 = mybir.dt.float32

# problem constants (hardcoded; kernel.py must be self-contained)
SAMPLE_RATE = 2048.0
FFTLENGTH = 8.0
NUM_BINS = 16
N = int(FFTLENGTH * SAMPLE_RATE)      # 16384
NF = N // 2 + 1                        # 8193
DF = 1.0 / FFTLENGTH
CSCALE = 4.0 * DF / (SAMPLE_RATE ** 2)

NCORES = 8
ROWS = 128            # rows per core (1024 total)
R = 4                 # rows per chunk
NCHUNK = ROWS // R    # 32
NK2 = 65              # k2 in [0, 64]
NPAD = 128 * NK2      # 8320


def _make_consts():
    n1 = np.arange(128)
    ang1 = 2 * np.pi * np.outer(n1, n1) / 128.0
    c128 = np.cos(ang1).astype(np.float32)         # symmetric
    s128 = np.sin(ang1).astype(np.float32)
    k1 = np.arange(128)
    n2 = np.arange(128)
    angw = 2 * np.pi * np.outer(k1, n2) / float(N)
    wr = np.cos(angw).astype(np.float32)           # TW = wr + i*wi
    wi = (-np.sin(angw)).astype(np.float32)
    wr_rep = np.tile(wr, (1, R)).astype(np.float32)
    wi_rep = np.tile(wi, (1, R)).astype(np.float32)
    k2 = np.arange(NK2)
    ange = 2 * np.pi * np.outer(n2, k2) / 128.0
    sc = np.float32(np.sqrt(CSCALE))
    er = (np.cos(ange) * sc).astype(np.float32)    # E = er + i*ei, pre-scaled
    ei = (-np.sin(ange) * sc).astype(np.float32)
    eineg = (-ei).astype(np.float32)
    ident = np.eye(128, dtype=np.float32)
    mfrac = np.tile((np.arange(1, 16, dtype=np.float32) / 16.0)[None, :], (128, 1))
    return dict(c128=c128, s128=s128, wr_rep=wr_rep, wi_rep=wi_rep,
                er=er, ei=ei, eineg=eineg, ident=ident, mfrac=mfrac)


def _build_program():
    nc = bacc.Bacc("TRN2", target_bir_lowering=False, debug=False,
                   enable_asserts=False, num_devices=NCORES)
    t_in = nc.dram_tensor("t_in", [ROWS, N], F32, kind="ExternalInput").ap()
    s_in = nc.dram_tensor("s_in", [ROWS, N], F32, kind="ExternalInput").ap()
    consts = _make_consts()
    capz = {k: nc.dram_tensor(k, list(v.shape), F32, kind="ExternalInput").ap()
            for k, v in consts.items()}
    out = nc.dram_tensor("chisq_out", [ROWS, 1], F32, kind="ExternalOutput").ap()

    AL = mybir.AluOpType

    with tile.TileContext(nc, trace_sim=False) as tc, ExitStack() as ctx:
        cpool = ctx.enter_context(tc.tile_pool(name="consts", bufs=1))
        big = ctx.enter_context(tc.tile_pool(name="big", bufs=1))
        inp = ctx.enter_context(tc.tile_pool(name="inp", bufs=2))
        work = ctx.enter_context(tc.tile_pool(name="work", bufs=1))
        ps1 = ctx.enter_context(tc.tile_pool(name="ps1", bufs=1, space="PSUM"))
        ps2 = ctx.enter_context(tc.tile_pool(name="ps2", bufs=1, space="PSUM"))
        ps3 = ctx.enter_context(tc.tile_pool(name="ps3", bufs=1, space="PSUM"))

        ct = {}
        for k, v in consts.items():
            ct[k] = cpool.tile(list(v.shape), F32, tag=k, name=k)
            nc.sync.dma_start(ct[k][:], capz[k][:])

        PH = big.tile([128, NPAD], F32, tag="PH")
        CR = big.tile([128, NPAD], F32, tag="CR")
        CH = big.tile([128, NPAD], F32, tag="CH")

        def fft_signal(x_dram, r0, out_psum, sig):
            """FFT chunk rows [r0, r0+R) of one signal.
            Returns (Xr_ps, Xi_ps) PSUM tiles [65, R*128] on grid [k2,(row,k1)]."""
            xt = inp.tile([128, R * 128], F32, tag="xt_" + sig, name="xt_" + sig)
            nc.sync.dma_start(
                xt[:].rearrange("p (r f) -> p r f", r=R),
                x_dram[r0:r0 + R, :].rearrange("r (p f) -> p r f", p=128))
            yc = ps1.tile([128, R * 128], F32, tag="yc_" + sig, name="yc_" + sig)
            ys = ps1.tile([128, R * 128], F32, tag="ys_" + sig, name="ys_" + sig)
            nc.tensor.matmul(yc[:], ct["c128"][:], xt[:], start=True, stop=True)
            nc.tensor.matmul(ys[:], ct["s128"][:], xt[:], start=True, stop=True)
            # twiddle: Zr = yc*wr + ys*wi ; Zi = yc*wi - ys*wr
            t1 = work.tile([128, R * 128], F32, tag="t1" + sig, name="t1" + sig)
            t2 = work.tile([128, R * 128], F32, tag="t2" + sig, name="t2" + sig)
            t3 = work.tile([128, R * 128], F32, tag="t3" + sig, name="t3" + sig)
            t4 = work.tile([128, R * 128], F32, tag="t4" + sig, name="t4" + sig)
            zr = work.tile([128, R * 128], F32, tag="zr" + sig, name="zr" + sig)
            zi = work.tile([128, R * 128], F32, tag="zi" + sig, name="zi" + sig)
            nc.vector.tensor_tensor(t1[:], yc[:], ct["wr_rep"][:], op=AL.mult)
            nc.vector.tensor_tensor(t2[:], ys[:], ct["wi_rep"][:], op=AL.mult)
            nc.gpsimd.tensor_tensor(zr[:], t1[:], t2[:], op=AL.add)
            nc.vector.tensor_tensor(t3[:], yc[:], ct["wi_rep"][:], op=AL.mult)
            nc.vector.tensor_tensor(t4[:], ys[:], ct["wr_rep"][:], op=AL.mult)
            nc.gpsimd.tensor_tensor(zi[:], t3[:], t4[:], op=AL.subtract)
            # transpose each row's [k1, n2] block -> [n2, k1]
            zrt = ps2.tile([128, R * 128], F32, tag="zrt", name="zrt" + sig)
            zit = ps2.tile([128, R * 128], F32, tag="zit", name="zit" + sig)
            for r in range(R):
                sl = slice(r * 128, (r + 1) * 128)
                nc.tensor.transpose(zrt[:, sl], zr[:, sl], ct["ident"][:])
                nc.tensor.transpose(zit[:, sl], zi[:, sl], ct["ident"][:])
            zrt_sb = work.tile([128, R * 128], F32, tag="zrt_sb" + sig, name="zrt_sb" + sig)
            zit_sb = work.tile([128, R * 128], F32, tag="zit_sb" + sig, name="zit_sb" + sig)
            nc.scalar.copy(zrt_sb[:], zrt[:])
            nc.scalar.copy(zit_sb[:], zit[:])
            # stage 3: X = Z @ E  (complex), out [k2, (row,k1)]
            xr = out_psum.tile([NK2, R * 128], F32, tag="xr", name="xr" + sig)
            xi = out_psum.tile([NK2, R * 128], F32, tag="xi", name="xi" + sig)
            nc.tensor.matmul(xr[:], ct["er"][:], zrt_sb[:], start=True, stop=False)
            nc.tensor.matmul(xr[:], ct["eineg"][:], zit_sb[:], start=False, stop=True)
            nc.tensor.matmul(xi[:], ct["ei"][:], zrt_sb[:], start=True, stop=False)
            nc.tensor.matmul(xi[:], ct["er"][:], zit_sb[:], start=False, stop=True)
            return xr, xi

        for ci in range(NCHUNK):
            r0 = ci * R
            xrt, xit = fft_signal(t_in, r0, ps3, "t")
            xrt_sb = work.tile([NK2, R * 128], F32, tag="xrt_sb", name="xrt_sb")
            xit_sb = work.tile([NK2, R * 128], F32, tag="xit_sb", name="xit_sb")
            nc.scalar.copy(xrt_sb[:], xrt[:])
            nc.scalar.copy(xit_sb[:], xit[:])
            xrs, xis = fft_signal(s_in, r0, ps3, "s")
            # ph = xrt^2 + xit^2 ; cross = xrt*xrs + xit*xis   (c-scaled via E)
            u1 = work.tile([NK2, R * 128], F32, tag="u1", name="u1")
            u2 = work.tile([NK2, R * 128], F32, tag="u2", name="u2")
            u3 = work.tile([NK2, R * 128], F32, tag="u3", name="u3")
            u4 = work.tile([NK2, R * 128], F32, tag="u4", name="u4")
            phc = work.tile([NK2, R * 128], F32, tag="phc", name="phc")
            crc = work.tile([NK2, R * 128], F32, tag="crc", name="crc")
            nc.scalar.square(u1[:], xrt_sb[:])
            nc.scalar.square(u2[:], xit_sb[:])
            nc.gpsimd.tensor_tensor(phc[:], u1[:], u2[:], op=AL.add)
            nc.vector.tensor_tensor(u3[:], xrt_sb[:], xrs[:], op=AL.mult)
            nc.vector.tensor_tensor(u4[:], xit_sb[:], xis[:], op=AL.mult)
            nc.gpsimd.tensor_tensor(crc[:], u3[:], u4[:], op=AL.add)
            # unflatten to [row, k] big tiles
            for r in range(R):
                sl = slice(r * 128, (r + 1) * 128)
                row = r0 + r
                nc.sync.dma_start(
                    PH[row:row + 1, :].rearrange("o (k f) -> o k f", k=NK2),
                    phc[:, sl].rearrange("k f -> k () f"))
                nc.sync.dma_start(
                    CR[row:row + 1, :].rearrange("o (k f) -> o k f", k=NK2),
                    crc[:, sl].rearrange("k f -> k () f"))

        # ---- binning ----
        nc.vector.memset(PH[:, NF:NPAD], 0.0)
        nc.vector.memset(CR[:, NF:NPAD], 0.0)
        nc.vector.tensor_tensor_scan(CH[:], PH[:], PH[:], 0.0, AL.add, AL.bypass)
        th = CH[:, NF - 1:NF]
        tvals = cpool.tile([128, 15], F32, tag="tvals")
        nc.vector.tensor_scalar(tvals[:], ct["mfrac"][:], th, None, op0=AL.mult)
        stot = cpool.tile([128, 1], F32, tag="stot")
        nc.vector.tensor_reduce(stot[:], CR[:, 0:NF], op=AL.add,
                                axis=mybir.AxisListType.X)
        G = cpool.tile([128, 17], F32, tag="G")
        nc.vector.memset(G[:], 0.0)
        nc.vector.tensor_tensor(G[:, 0:1], stot[:], CR[:, 0:1], op=AL.subtract)
        scratch = big.tile([128, NPAD - 1], F32, tag="scratch")
        for m in range(1, 16):
            nc.vector.scalar_tensor_tensor(
                scratch[:, 0:NPAD - 1], CH[:, 0:NPAD - 1], tvals[:, m - 1:m],
                CR[:, 1:NPAD], AL.is_gt, AL.mult,
                accum_out=G[:, m:m + 1])
        snrb = cpool.tile([128, 16], F32, tag="snrb")
        nc.vector.tensor_tensor(snrb[:], G[:, 0:16], G[:, 1:17], op=AL.subtract)
        s16 = cpool.tile([128, 1], F32, tag="s16")
        nc.vector.tensor_scalar_mul(s16[:], stot[:], 1.0 / 16.0)
        ee = cpool.tile([128, 16], F32, tag="ee")
        nc.vector.tensor_scalar(ee[:], snrb[:], s16[:], None, op0=AL.subtract)
        esq = cpool.tile([128, 16], F32, tag="esq")
        nc.vector.tensor_tensor(esq[:], ee[:], ee[:], op=AL.mult)
        ssum = cpool.tile([128, 1], F32, tag="ssum")
        nc.vector.tensor_reduce(ssum[:], esq[:], op=AL.add,
                                axis=mybir.AxisListType.X)
        rth = cpool.tile([128, 1], F32, tag="rth")
        nc.vector.reciprocal(rth[:], th)
        chq = cpool.tile([128, 1], F32, tag="chq")
        nc.vector.tensor_tensor(chq[:], ssum[:], rth[:], op=AL.mult)
        nc.vector.tensor_scalar_mul(chq[:], chq[:], float(NUM_BINS) / (NUM_BINS - 1))
        nc.sync.dma_start(out[:], chq[:])

    nc.compile()
    return nc, consts


_CACHE = {}


def kernel(template: np.ndarray, strain: np.ndarray) -> np.ndarray:
    if "nc" not in _CACHE:
        _CACHE["nc"], _CACHE["consts"] = _build_program()
    nc, consts = _CACHE["nc"], _CACHE["consts"]

    t = np.ascontiguousarray(np.asarray(template, np.float32).reshape(1024, N))
    s = np.ascontiguousarray(np.asarray(strain, np.float32).reshape(1024, N))
    in_maps = []
    for c in range(NCORES):
        m = {"t_in": t[c * ROWS:(c + 1) * ROWS], "s_in": s[c * ROWS:(c + 1) * ROWS]}
        m.update(consts)
        in_maps.append(m)
    res = run_bass_kernel_spmd(nc, in_maps, list(range(NCORES)))
    outs = [res.results[c]["chisq_out"].reshape(ROWS) for c in range(NCORES)]
    full = np.concatenate(outs).astype(np.float32)
    return full.reshape(512, 2)


if __name__ == "__main__":
    rng = np.random.default_rng(0)
    tpl = rng.standard_normal((512, 2, N), dtype=np.float32)
    st = rng.standard_normal((512, 2, N), dtype=np.float32)
    print(kernel(tpl, st)[:3])



# revision 14
# speedup vs baseline: 2.5255x; 2.5255x over previous
"""Trainium2 Bass kernel for the ChiSq (histogram_binning) problem.

Per core (128 of 1024 rows, pure data parallel):
  FFT-16384 of template & strain via 2-stage radix-128 matmul FFT with the
  data as the stationary operand (no transposes), Hermitian-halved twiddle
  (k1 <= 64 only; upper half reconstructed inside stage-3 via a conjugated
  E-matrix "set B"), bf16 moving operands for full-rate PE.
  Binning is hierarchical: per-128 block sums -> block-level masked sums on
  [128, 65] arrays, plus an indirect-DMA gather of the one straddling block
  per (row, threshold) for the exact fine correction. This avoids both the
  big row-major unflatten DMAs and the 15 full-array masked passes.
"""
import numpy as np
from contextlib import ExitStack

import concourse.bass as bass
import concourse.tile as tile
from concourse import bacc, mybir
from concourse.bass_utils import run_bass_kernel_spmd

F32 = mybir.dt.float32
F32R = mybir.dt.float32r
BF16 = mybir.dt.bfloat16
I32 = mybir.dt.int32

SAMPLE_RATE = 2048.0
FFTLENGTH = 8.0
NUM_BINS = 16
N = int(FFTLENGTH * SAMPLE_RATE)       # 16384
NF = N // 2 + 1                        # 8193
DF = 1.0 / FFTLENGTH
CSCALE = 4.0 * DF / (SAMPLE_RATE ** 2)

NCORES = 8
ROWS = 128          # rows per core
GROUPS = 8
GR = 16             # rows per group (DMA granularity)
R = 4               # rows per compute chunk
CPG = GR // R       # chunks per group
NB = 65             # 64 full blocks + tail block (k = 8192)
PITCH = 66          # DRAM row pitch in 128-blocks (alignment pad)
NM = 16             # thresholds m = 0..15 (t_0 = 0)


def _make_consts():
    s = np.sqrt(np.float32(CSCALE))
    n1 = np.arange(128)
    k1 = np.arange(128)
    ang1 = 2 * np.pi * np.outer(n1, k1) / 128.0
    CS = np.concatenate([np.cos(ang1), np.sin(ang1)], axis=1).astype(np.float32)

    n2 = np.arange(128)
    j = np.arange(65)
    angw = 2 * np.pi * np.outer(n2, j) / float(N)
    twr = np.cos(angw)
    twi = -np.sin(angw)
    twr_rep = np.tile(twr, (1, R))    # [128, 260] (r, j) blocks
    twi_rep = np.tile(twi, (1, R))

    k2 = np.arange(65)
    angA = 2 * np.pi * np.outer(n2, k2) / 128.0
    erA = np.cos(angA) * s
    eipA = np.sin(angA) * s
    einA = -eipA
    k2b = np.arange(64)
    angB = 2 * np.pi * np.outer(n2, k2b + 1) / 128.0
    ebr = np.cos(angB) * s
    ebi = np.sin(angB) * s
    ebin = -ebi
    cbf = np.concatenate(
        [twr_rep, twi_rep, erA, eipA, einA, ebr, ebi, ebin], axis=1)  # [128, 907]

    ident = np.eye(128, dtype=np.float32)
    mfrac = np.tile((np.arange(16, dtype=np.float32) / 16.0)[None, :], (128, 1))
    row66 = (np.arange(128, dtype=np.float32) * PITCH)[:, None]
    ones65 = np.ones((128, 65), dtype=np.float32)
    cf = np.concatenate([ident, mfrac, row66, ones65], axis=1)  # [128, 210]

    import ml_dtypes
    return dict(
        cs=np.ascontiguousarray(CS, np.float32),
        cbf=np.ascontiguousarray(cbf).astype(ml_dtypes.bfloat16),
        cf=np.ascontiguousarray(cf, np.float32),
    )


def _build_program():
    nc = bacc.Bacc("TRN2", target_bir_lowering=False, debug=False,
                   enable_asserts=False, num_devices=NCORES)
    t_in = nc.dram_tensor("t_in", [ROWS, N], F32R, kind="ExternalInput").ap()
    s_in = nc.dram_tensor("s_in", [ROWS, N], F32R, kind="ExternalInput").ap()
    cs_d = nc.dram_tensor("cs", [128, 256], F32R, kind="ExternalInput").ap()
    cbf_d = nc.dram_tensor("cbf", [128, 907], BF16, kind="ExternalInput").ap()
    cf_d = nc.dram_tensor("cf", [128, 210], F32, kind="ExternalInput").ap()
    # interleaved staging table: [..., 0:128] = ph block, [..., 128:256] = cr
    tab_d = nc.dram_tensor("tab_d", [ROWS, PITCH, 256], BF16, kind="Internal").ap()
    out = nc.dram_tensor("chisq_out", [ROWS, 1], F32, kind="ExternalOutput").ap()

    AL = mybir.AluOpType
    AX = mybir.AxisListType

    with tile.TileContext(nc, trace_sim=False) as tc, ExitStack() as ctx:
        cpool = ctx.enter_context(tc.tile_pool(name="consts", bufs=1))
        inp = ctx.enter_context(tc.tile_pool(name="inp", bufs=2))
        ybfp = ctx.enter_context(tc.tile_pool(name="ybfp", bufs=2))
        zp = ctx.enter_context(tc.tile_pool(name="zp", bufs=2))
        up = ctx.enter_context(tc.tile_pool(name="up", bufs=2))
        xbp = ctx.enter_context(tc.tile_pool(name="xbp", bufs=2))
        tqp = ctx.enter_context(tc.tile_pool(name="tqp", bufs=2))
        gp = ctx.enter_context(tc.tile_pool(name="gp", bufs=2))
        persist = ctx.enter_context(tc.tile_pool(name="persist", bufs=1))
        fin = ctx.enter_context(tc.tile_pool(name="fin", bufs=1))
        psy = ctx.enter_context(tc.tile_pool(name="psy", bufs=2, space="PSUM"))
        psx = ctx.enter_context(tc.tile_pool(name="psx", bufs=2, space="PSUM"))

        csr = cpool.tile([128, 256], F32R, tag="csr", name="csr")
        cbf = cpool.tile([128, 907], BF16, tag="cbf", name="cbf")
        cf = cpool.tile([128, 210], F32, tag="cf", name="cf")
        nc.sync.dma_start(csr[:], cs_d[:])
        nc.sync.dma_start(cbf[:], cbf_d[:])
        nc.sync.dma_start(cf[:], cf_d[:])
        twr_v = cbf[:, 0:260]
        twi_v = cbf[:, 260:520]
        erA = cbf[:, 520:585]
        eipA = cbf[:, 585:650]
        einA = cbf[:, 650:715]
        ebr = cbf[:, 715:779]
        ebi = cbf[:, 779:843]
        ebin = cbf[:, 843:907]
        ident = cf[:, 0:128]
        mfrac = cf[:, 128:144]
        row66 = cf[:, 144:145]
        ones65 = cf[:, 145:210]

        # zero-pad the invalid tail of block 64 ([8193, 8320) = junk) and the
        # alignment pad block 65 (never gathered, but keep DRAM defined)
        zt = cpool.tile([128, 256], BF16, tag="zt", name="zt")
        nc.vector.memset(zt[:], 0.0)
        nc.sync.dma_start(tab_d[:, 64:65, 1:128], zt[:, 0:127])
        nc.sync.dma_start(tab_d[:, 64:65, 129:256], zt[:, 0:127])
        nc.sync.dma_start(tab_d[:, 65:66, :], zt[:])

        # persistent block-sum accumulators, [block, row] layout
        bsh_t = persist.tile([65, 128], F32, tag="bsh_t", name="bsh_t")
        bsc_t = persist.tile([65, 128], F32, tag="bsc_t", name="bsc_t")

        def fft_half(xg, cl, sig):
            """Stage 1 + twiddle for chunk cl of group tile xg (one signal).
            Returns (zr, zi) bf16 [128, (R,65)] = Z[n2, k1<=64] per row."""
            ya = psy.tile([128, 512], F32, tag="ya", name="ya_" + sig)
            yb = psy.tile([128, 512], F32, tag="yb", name="yb_" + sig)
            for rl in range(R):
                xrow = xg[:, (cl * R + rl) * 128:(cl * R + rl + 1) * 128]
                yt = ya if rl < 2 else yb
                c0 = (rl % 2) * 256
                nc.tensor.matmul(yt[:, c0:c0 + 256], xrow, csr[:],
                                 start=True, stop=True)
            ybf = ybfp.tile([128, R * 130], BF16, tag="ybf", name="ybf_" + sig)
            for hi, yt in ((0, ya), (1, yb)):
                ysrc = yt[:].rearrange("p (r t f) -> p r t f", t=2, f=128)[:, :, :, 0:65]
                nc.scalar.copy(
                    ybf[:, hi * 260:(hi + 1) * 260].rearrange(
                        "p (r t f) -> p r t f", t=2, f=65), ysrc)
            ycb = ybf[:].rearrange("p (r t f) -> p r t f", t=2, f=65)[:, :, 0, :]
            ysb = ybf[:].rearrange("p (r t f) -> p r t f", t=2, f=65)[:, :, 1, :]
            u1 = up.tile([128, R * 65], BF16, tag="u1", name="u1_" + sig)
            u2 = up.tile([128, R * 65], BF16, tag="u2", name="u2_" + sig)
            u3 = up.tile([128, R * 65], BF16, tag="u3", name="u3_" + sig)
            u4 = up.tile([128, R * 65], BF16, tag="u4", name="u4_" + sig)
            zr = zp.tile([128, R * 65], BF16, tag="zr", name="zr_" + sig)
            zi = zp.tile([128, R * 65], BF16, tag="zi", name="zi_" + sig)
            tw_r = twr_v.rearrange("p (r f) -> p r f", f=65)
            tw_i = twi_v.rearrange("p (r f) -> p r f", f=65)
            nc.vector.scalar_tensor_tensor(
                u1[:].rearrange("p (r f) -> p r f", f=65), ycb, 0.0, tw_r,
                op0=AL.bypass, op1=AL.mult)
            nc.vector.scalar_tensor_tensor(
                u2[:].rearrange("p (r f) -> p r f", f=65), ysb, 0.0, tw_i,
                op0=AL.bypass, op1=AL.mult)
            nc.vector.tensor_tensor(zr[:], u1[:], u2[:], op=AL.add)
            nc.vector.scalar_tensor_tensor(
                u3[:].rearrange("p (r f) -> p r f", f=65), ycb, 0.0, tw_i,
                op0=AL.bypass, op1=AL.mult)
            nc.vector.scalar_tensor_tensor(
                u4[:].rearrange("p (r f) -> p r f", f=65), ysb, 0.0, tw_r,
                op0=AL.bypass, op1=AL.mult)
            nc.vector.tensor_tensor(zi[:], u3[:], u4[:], op=AL.subtract)
            return zr, zi

        def stage3(zr, zi, sig):
            """Complex stage-3 DFT over n2. Returns bf16 SBUF tiles:
            (ar, ai) [65, (R,65)] for k1 in [0,64], and (br, bi) [64, (R,65)]
            whose col j maps to k1 = 128 - j (j=0 and j=64 are junk)."""
            xr = psx.tile([65, R * 65], F32, tag="xr", name="xrA_" + sig)
            xi = psx.tile([65, R * 65], F32, tag="xi", name="xiA_" + sig)
            nc.tensor.matmul(xr[:], erA, zr[:], start=True, stop=False)
            nc.tensor.matmul(xr[:], eipA, zi[:], start=False, stop=True)
            nc.tensor.matmul(xi[:], einA, zr[:], start=True, stop=False)
            nc.tensor.matmul(xi[:], erA, zi[:], start=False, stop=True)
            ar = xbp.tile([65, R * 65], BF16, tag="ar", name="arA_" + sig)
            ai = xbp.tile([65, R * 65], BF16, tag="ai", name="aiA_" + sig)
            nc.scalar.copy(ar[:], xr[:])
            nc.scalar.copy(ai[:], xi[:])
            xrb = psx.tile([65, R * 65], F32, tag="xr", name="xrB_" + sig)
            xib = psx.tile([65, R * 65], F32, tag="xi", name="xiB_" + sig)
            nc.tensor.matmul(xrb[0:64, :], ebr, zr[:], start=True, stop=False)
            nc.tensor.matmul(xrb[0:64, :], ebin, zi[:], start=False, stop=True)
            nc.tensor.matmul(xib[0:64, :], ebi, zr[:], start=True, stop=False)
            nc.tensor.matmul(xib[0:64, :], ebr, zi[:], start=False, stop=True)
            br = xbp.tile([65, R * 65], BF16, tag="br", name="brB_" + sig)
            bi = xbp.tile([65, R * 65], BF16, tag="bi", name="biB_" + sig)
            nc.scalar.copy(br[0:64, :], xrb[0:64, :])
            nc.scalar.copy(bi[0:64, :], xib[0:64, :])
            return ar, ai, br, bi

        def pair_prod_sum(dest, e0, e1, f0, f1, parts, rev, eng):
            """dest = e0*e1 + f0*f1 elementwise (bf16), written through
            3D views; rev reverses the j-axis of the sources (set B)."""
            t1 = tqp.tile([65, R * 65], BF16, tag="t1", name="pp1")
            t2 = tqp.tile([65, R * 65], BF16, tag="t2", name="pp2")
            eng.scalar_tensor_tensor(t1[0:parts, :], e0[0:parts, :], 0.0,
                                     e1[0:parts, :], op0=AL.bypass, op1=AL.mult)
            eng.scalar_tensor_tensor(t2[0:parts, :], f0[0:parts, :], 0.0,
                                     f1[0:parts, :], op0=AL.bypass, op1=AL.mult)
            if rev:
                s1 = t1[0:parts, :].rearrange("p (r f) -> p r f", f=65)[:, :, 63:0:-1]
                s2 = t2[0:parts, :].rearrange("p (r f) -> p r f", f=65)[:, :, 63:0:-1]
            else:
                s1 = t1[0:parts, :].rearrange("p (r f) -> p r f", f=65)
                s2 = t2[0:parts, :].rearrange("p (r f) -> p r f", f=65)
            eng.tensor_tensor(dest, s1, s2, op=AL.add)

        for g in range(GROUPS):
            xg_t = inp.tile([128, GR * 128], F32R, tag="xg_t", name="xg_t")
            xg_s = inp.tile([128, GR * 128], F32R, tag="xg_s", name="xg_s")
            nc.sync.dma_start(
                xg_t[:].rearrange("p (r f) -> p r f", r=GR),
                t_in[g * GR:(g + 1) * GR, :].rearrange("r (p f) -> p r f", p=128))
            nc.sync.dma_start(
                xg_s[:].rearrange("p (r f) -> p r f", r=GR),
                s_in[g * GR:(g + 1) * GR, :].rearrange("r (p f) -> p r f", p=128))
            gph = gp.tile([65, GR * 128], BF16, tag="gph", name="gph")
            gcr = gp.tile([65, GR * 128], BF16, tag="gcr", name="gcr")
            gph3 = gph[:].rearrange("p (r f) -> p r f", f=128)
            gcr3 = gcr[:].rearrange("p (r f) -> p r f", f=128)
            for cl in range(CPG):
                ci = g * CPG + cl           # global chunk id [0, 32)
                zr_t, zi_t = fft_half(xg_t, cl, "t")
                ar_t, ai_t, br_t, bi_t = stage3(zr_t, zi_t, "t")
                zr_s, zi_s = fft_half(xg_s, cl, "s")
                ar_s, ai_s, br_s, bi_s = stage3(zr_s, zi_s, "s")
                r0, r1 = cl * R, (cl + 1) * R
                # ph = |X_t|^2 ; cr = Re(conj(X_t) X_s)  (both c-scaled)
                pair_prod_sum(gph3[:, r0:r1, 0:65], ar_t, ar_t, ai_t, ai_t,
                              65, False, nc.vector)
                pair_prod_sum(gph3[0:64, r0:r1, 65:128], br_t, br_t, bi_t, bi_t,
                              64, True, nc.vector)
                pair_prod_sum(gcr3[:, r0:r1, 0:65], ar_t, ar_s, ai_t, ai_s,
                              65, False, nc.vector)
                pair_prod_sum(gcr3[0:64, r0:r1, 65:128], br_t, br_s, bi_t, bi_s,
                              64, True, nc.vector)
                # block sums (full blocks 0..63); tail block = single element
                nc.vector.tensor_reduce(
                    bsh_t[0:64, ci * R:(ci + 1) * R], gph3[0:64, r0:r1, :],
                    op=AL.add, axis=AX.X)
                nc.vector.tensor_reduce(
                    bsc_t[0:64, ci * R:(ci + 1) * R], gcr3[0:64, r0:r1, :],
                    op=AL.add, axis=AX.X)
                nc.scalar.copy(bsh_t[64:65, ci * R:(ci + 1) * R],
                               gph[64:65, r0 * 128:r1 * 128:128])
                nc.scalar.copy(bsc_t[64:65, ci * R:(ci + 1) * R],
                               gcr[64:65, r0 * 128:r1 * 128:128])
            # stage this group's ph/cr to DRAM, row-major with pitch 66 blocks
            nc.sync.dma_start(
                tab_d[g * GR:(g + 1) * GR, 0:64, 0:128].rearrange("r b j -> b r j"),
                gph[0:64, :].rearrange("p (r j) -> p r j", j=128))
            nc.sync.dma_start(
                tab_d[g * GR:(g + 1) * GR, 0:64, 128:256].rearrange("r b j -> b r j"),
                gcr[0:64, :].rearrange("p (r j) -> p r j", j=128))

        # ---- tail-block (k = 8192) values to DRAM ----
        tbh = fin.tile([1, 128], BF16, tag="tbh", name="tbh")
        tbc = fin.tile([1, 128], BF16, tag="tbc", name="tbc")
        nc.vector.tensor_copy(tbh[:], bsh_t[64:65, :])
        nc.vector.tensor_copy(tbc[:], bsc_t[64:65, :])
        nc.sync.dma_start(tab_d[:, 64:65, 0:1].rearrange("r b j -> b r j"), tbh[:])
        nc.sync.dma_start(tab_d[:, 64:65, 128:129].rearrange("r b j -> b r j"), tbc[:])

        # ---- transpose block sums to [row, block] ----
        trpa = psy.tile([128, 512], F32, tag="ya", name="tr_psa")
        trpb = psy.tile([128, 512], F32, tag="yb", name="tr_psb")
        nc.tensor.transpose(trpa[:, 0:65], bsh_t[:], ident[0:65, 0:65])
        nc.tensor.transpose(trpb[:, 0:65], bsc_t[:], ident[0:65, 0:65])
        bsh = fin.tile([128, 65], F32, tag="bsh", name="bsh")
        bsc = fin.tile([128, 65], F32, tag="bsc", name="bsc")
        nc.scalar.copy(bsh[:], trpa[:, 0:65])
        nc.scalar.copy(bsc[:], trpb[:, 0:65])
        chb = fin.tile([128, 65], F32, tag="chb", name="chb")
        cqb = fin.tile([128, 65], F32, tag="cqb", name="cqb")
        nc.vector.tensor_tensor_scan(chb[:], bsh[:], bsh[:], 0.0,
                                     op0=AL.add, op1=AL.bypass)
        nc.vector.tensor_tensor_scan(cqb[:], bsc[:], bsc[:], 0.0,
                                     op0=AL.add, op1=AL.bypass)
        th = chb[:, 64:65]     # total_h
        tc_ = cqb[:, 64:65]    # total_c
        tvals = fin.tile([128, NM], F32, tag="tvals", name="tvals")
        nc.vector.tensor_scalar(tvals[:], mfrac, th, None, op0=AL.mult)

        # ---- coarse masked sums over blocks ----
        bstar = fin.tile([128, NM], F32, tag="bstar", name="bstar")
        acc_a = fin.tile([128, NM], F32, tag="acc_a", name="acc_a")
        acc_p = fin.tile([128, NM], F32, tag="acc_p", name="acc_p")
        junk_v = fin.tile([128, 65], F32, tag="junk_v", name="junk_v")
        junk_g = fin.tile([128, 65], F32, tag="junk_g", name="junk_g")
        for m in range(NM):
            sc = tvals[:, m:m + 1]
            nc.vector.scalar_tensor_tensor(
                junk_v[:], chb[:], sc, ones65, op0=AL.is_le, op1=AL.mult,
                accum_out=bstar[:, m:m + 1])
            nc.vector.scalar_tensor_tensor(
                junk_g[:], chb[:], sc, bsc[:], op0=AL.is_le, op1=AL.mult,
                accum_out=acc_a[:, m:m + 1])
            nc.vector.scalar_tensor_tensor(
                junk_g[:], chb[:], sc, bsh[:], op0=AL.is_le, op1=AL.mult,
                accum_out=acc_p[:, m:m + 1])
        tau = fin.tile([128, NM], F32, tag="tau", name="tau")
        nc.vector.tensor_tensor(tau[:], tvals[:], acc_p[:], op=AL.subtract)
        idxf = fin.tile([128, NM], F32, tag="idxf", name="idxf")
        nc.vector.tensor_scalar(idxf[:], bstar[:], row66, None, op0=AL.add)
        idx = fin.tile([128, NM], I32, tag="idx", name="idx")
        nc.vector.tensor_copy(idx[:], idxf[:])

        # ---- gather straddling blocks (interleaved ph|cr windows) ----
        # HW DGE semantics: one offset per partition, consecutive streaming —
        # so issue one indirect DMA per threshold m.
        wins = fin.tile([128, NM * 256], BF16, tag="wins", name="wins")
        tab_flat = tab_d[:].rearrange("r b j -> (r b) j")
        for m in range(NM):
            nc.gpsimd.indirect_dma_start(
                wins[:, m * 256:(m + 1) * 256], None, tab_flat,
                bass.IndirectOffsetOnAxis(ap=idx[:, m:m + 1], axis=0))

        # ---- fine correction: F_m = sum_j [CHprev <= t] cr within block ----
        loc = fin.tile([128, NM * 128], F32, tag="loc", name="loc")
        f1 = fin.tile([128, NM], F32, tag="f1", name="f1")
        junk2 = fin.tile([128, 127], F32, tag="junk2", name="junk2")
        for m in range(NM):
            phw = wins[:, m * 256:m * 256 + 128]
            crw = wins[:, m * 256 + 128:m * 256 + 256]
            sl = slice(m * 128, (m + 1) * 128)
            nc.vector.tensor_tensor_scan(
                loc[:, sl], phw, phw, 0.0, op0=AL.add, op1=AL.bypass)
            nc.vector.scalar_tensor_tensor(
                junk2[:], loc[:, m * 128:m * 128 + 127], tau[:, m:m + 1],
                crw[:, 1:128], op0=AL.is_le, op1=AL.mult,
                accum_out=f1[:, m:m + 1])
        crw0 = fin.tile([128, NM], F32, tag="crw0", name="crw0")
        nc.vector.tensor_copy(crw0[:], wins[:, 128:NM * 256:256])

        # negG[m] = A_m + F_m - total_c  (so snr_bin = negG[m+1] - negG[m])
        negg = fin.tile([128, NM + 1], F32, tag="negg", name="negg")
        nc.vector.memset(negg[:, NM:NM + 1], 0.0)
        nc.vector.tensor_tensor(negg[:, 0:NM], acc_a[:], f1[:], op=AL.add)
        nc.vector.tensor_tensor(negg[:, 0:NM], negg[:, 0:NM], crw0[:], op=AL.add)
        nc.vector.tensor_scalar(negg[:, 0:NM], negg[:, 0:NM], tc_, None,
                                op0=AL.subtract)
        snr = fin.tile([128, NM], F32, tag="snr", name="snr")
        nc.vector.tensor_tensor(snr[:], negg[:, 1:NM + 1], negg[:, 0:NM],
                                op=AL.subtract)
        s16 = fin.tile([128, 1], F32, tag="s16", name="s16")
        nc.vector.tensor_scalar_mul(s16[:], tc_, 1.0 / NUM_BINS)
        ee = fin.tile([128, NM], F32, tag="ee", name="ee")
        nc.vector.tensor_scalar(ee[:], snr[:], s16[:], None, op0=AL.subtract)
        esq = fin.tile([128, NM], F32, tag="esq", name="esq")
        nc.vector.tensor_tensor(esq[:], ee[:], ee[:], op=AL.mult)
        ssum = fin.tile([128, 1], F32, tag="ssum", name="ssum")
        nc.vector.tensor_reduce(ssum[:], esq[:], op=AL.add, axis=AX.X)
        rth = fin.tile([128, 1], F32, tag="rth", name="rth")
        nc.vector.reciprocal(rth[:], th)
        chq = fin.tile([128, 1], F32, tag="chq", name="chq")
        nc.vector.tensor_tensor(chq[:], ssum[:], rth[:], op=AL.mult)
        nc.vector.tensor_scalar_mul(chq[:], chq[:],
                                    float(NUM_BINS) / (NUM_BINS - 1))
        nc.sync.dma_start(out[:], chq[:])

    nc.compile()
    return nc, _make_consts()


_CACHE = {}


def kernel(template: np.ndarray, strain: np.ndarray) -> np.ndarray:
    if "nc" not in _CACHE:
        _CACHE["nc"], _CACHE["consts"] = _build_program()
    nc, consts = _CACHE["nc"], _CACHE["consts"]

    t = np.ascontiguousarray(np.asarray(template, np.float32).reshape(1024, N))
    s = np.ascontiguousarray(np.asarray(strain, np.float32).reshape(1024, N))
    in_maps = []
    for c in range(NCORES):
        m = {"t_in": t[c * ROWS:(c + 1) * ROWS],
             "s_in": s[c * ROWS:(c + 1) * ROWS]}
        m.update(consts)
        in_maps.append(m)
    res = run_bass_kernel_spmd(nc, in_maps, list(range(NCORES)))
    outs = [res.results[c]["chisq_out"].reshape(ROWS) for c in range(NCORES)]
    full = np.concatenate(outs).astype(np.float32)
    return full.reshape(512, 2)


if __name__ == "__main__":
    rng = np.random.default_rng(0)
    tpl = rng.standard_normal((512, 2, N), dtype=np.float32)
    st = rng.standard_normal((512, 2, N), dtype=np.float32)
    print(kernel(tpl, st)[:3])
